# revision 3
# baseline (speedup 1.0000x reference)
"""Trainium2 Bass kernel for the soft-decision-tree ensemble classifier (V4).

Restructure vs V3b: the deepest tree level is factored out of the log-domain
path matmul.  For parent node j (level-5 node 31+j), its two leaves satisfy
  lp[2j]   = exp(cm_j) * r5_j,      r5 = 1/(1+e^{z5})
  lp[2j+1] = exp(cm_j) * (1-r5_j)
with cm the level-0..4 path log-prob.  Folding into the output matmul with
V_A = V_odd, V_B = V_even - V_odd gives  out += V_A^T ep + V_B^T (ep*r5).
This halves the ScalarE exp/ln work (only 31 of 63 nodes need softplus), and
halves the A-matmul contraction (32-wide blocks, 4 trees per 128-tile).
The leaf-distribution softmax is computed on the host and DMA'd as V_A/V_B.

Sharding: 2-way trees x 4-way batch (unchanged).  Per core: 32 trees in 8
groups of 4, batch 1024 in 2 blocks of 512.  All matmuls fp16, N=512.
PSUM: pz pairs (sh|dp, 2 banks) x2, pp pair x1 (2 banks), out x2 = 8 banks.
Warm-up matmuls on a memset tile run during the DMA/preamble head so the PE
HAM clock-gate is released before real matmuls start.
"""

import numpy as np

TREE_DEPTH = 6
T, N, D, C = 64, 63, 512, 100
L = 2**TREE_DEPTH
TG = 2
BG = 4
TL = T // TG               # 32 trees per core
NG = 8                     # tree groups of 4 per core
B = 4096
NCORES = 8
BSL = B // BG              # 1024 batch rows per core
VB = 512

N_WARM = 32

_NC_CACHE = {}


def _parent_paths():
    """For each level-0..4 parent path j: the shallow nodes visited and the
    direction bit at each, checking the level-5 node is 31+j."""
    paths = []
    for j in range(32):
        node = 0
        steps = []
        for k in range(5):
            bit = (j >> (4 - k)) & 1
            steps.append((node, bit))
            node = 2 * node + 1 + bit
        assert node == 31 + j
        paths.append(steps)
    return paths


def _pack_amat():
    """[128, 256] fp16: [:, :128] = A'dir, [:, 128:] = A'path, block-diagonal
    over 4 trees of (32 shallow nodes x 32 parents)."""
    adir = np.zeros((32, 32), np.float32)
    apath = np.zeros((32, 32), np.float32)
    for j, steps in enumerate(_parent_paths()):
        for node, bit in steps:
            if bit:
                adir[node, j] += 1.0
            apath[node, j] -= 1.0
    amat = np.zeros((128, 256), np.float16)
    for r in range(4):
        sl = slice(32 * r, 32 * r + 32)
        amat[sl, sl] = adir
        amat[sl, 128 + 32 * r:128 + 32 * r + 32] = apath
    return amat


_AMAT = _pack_amat()


def _build_bass():
    import concourse.bacc as bacc
    import concourse.mybir as mybir
    import concourse.tile as tile
    from concourse.hw_specs import get_activation_tables

    dt = mybir.dt
    f32 = dt.float32
    f32r = dt.float32r
    fp16 = dt.float16
    AF = mybir.ActivationFunctionType
    ALU = mybir.AluOpType

    nc = bacc.Bacc("TRN2", target_bir_lowering=False, debug=False,
                   num_devices=NCORES)

    table_id = next(i for i, (_, funcs) in
                    enumerate(get_activation_tables("gen3").items())
                    if AF.Exp in funcs and AF.Ln in funcs)
    nc.scalar.add_instruction(mybir.InstLoadActFuncSet(
        name=f"I-{nc.next_id()}", ins=[], outs=[], act_func_set_id=table_id))

    # ---- DRAM tensors ------------------------------------------------
    xt = nc.dram_tensor("xt", [128, 4096], fp16, kind="ExternalInput").ap()
    wt = nc.dram_tensor("wt", [4096, 256], fp16, kind="ExternalInput").ap()
    consts = nc.dram_tensor("consts", [128, 32], f32r,
                            kind="ExternalInput").ap()
    amat = nc.dram_tensor("amat", [128, 256], fp16, kind="ExternalInput").ap()
    vmat = nc.dram_tensor("vmat", [128, 2048], fp16,
                          kind="ExternalInput").ap()
    outs = {}
    for v in range(2):
        for h in ("A", "B"):
            nm = f"o{h}{v}"
            outs[(v, h)] = nc.dram_tensor(nm, [100, VB], f32,
                                          kind="ExternalOutput").ap()

    with tile.TileContext(nc) as tc:
        with (
            tc.tile_pool(name="big", bufs=1) as bigp,
            tc.tile_pool(name="const", bufs=1) as constp,
            tc.tile_pool(name="ta", bufs=3) as tap,
            tc.tile_pool(name="work", bufs=2) as work,
            tc.tile_pool(name="pz", bufs=2, space="PSUM") as pzp,
            tc.tile_pool(name="pp", bufs=1, space="PSUM") as ppp,
            tc.tile_pool(name="po", bufs=2, space="PSUM") as pop,
        ):
            wt_t = bigp.tile([128, 8192], fp16, tag="wt")
            xt_t = bigp.tile([128, 4096], fp16, tag="xt")
            vm_t = bigp.tile([128, 2048], fp16, tag="vm")
            consts_t = constp.tile([128, 32], f32r, tag="consts")
            amat_t = constp.tile([128, 256], fp16, tag="amat")
            warm_t = constp.tile([128, 128], fp16, tag="warm")

            # ---- warm-up: memset tile + back-to-back matmuls ---------
            nc.gpsimd.memset(warm_t[:], 0.0)
            warm_ps = pop.tile([128, VB], f32, tag="ops", name="warm_ps")
            for _ in range(N_WARM):
                nc.tensor.matmul(warm_ps[:, 0:128], lhsT=warm_t[:],
                                 rhs=warm_t[:], start=True, stop=True)

            # ---- DMA plan --------------------------------------------
            k = 0

            def dma(out_, in_):
                nonlocal k
                eng = nc.gpsimd if k % 2 == 0 else nc.sync
                eng.dma_start(out=out_, in_=in_)
                k += 1

            def wt_piece(j, g0, g1):
                ng = g1 - g0
                dma(wt_t[:, j * 2048 + g0 * 256:j * 2048 + g1 * 256]
                    .rearrange("p (g c) -> p g c", c=256),
                    wt[j * 1024 + g0 * 128:j * 1024 + g1 * 128, :]
                    .rearrange("(g p) c -> p g c", p=128))

            def xt_piece(j, v):
                c0 = j * 1024 + v * VB
                dma(xt_t[:, c0:c0 + VB], xt[:, c0:c0 + VB])

            dma(amat_t[:], amat[:])
            dma(consts_t[:], consts[:])
            xt_piece(0, 0)
            wt_piece(0, 0, 2)
            wt_piece(1, 0, 2)
            xt_piece(1, 0)
            xt_piece(2, 0)
            wt_piece(2, 0, 2)
            wt_piece(3, 0, 2)
            xt_piece(3, 0)
            # bulk: rest of weights, vmat, x v1
            dma(vm_t[:, 0:512], vmat[:, 0:512])
            wt_piece(0, 2, 8)
            wt_piece(1, 2, 8)
            dma(vm_t[:, 512:1024], vmat[:, 512:1024])
            wt_piece(2, 2, 8)
            wt_piece(3, 2, 8)
            dma(vm_t[:, 1024:2048], vmat[:, 1024:2048])
            for j in range(4):
                xt_piece(j, 1)

            adir_ap = amat_t[:, 0:128]
            apath_ap = amat_t[:, 128:256]

            def bias_bc(g):
                return (consts_t[:, 2 * g:2 * g + 2].bitcast(f32)
                        .unsqueeze(2).broadcast_to([128, 2, VB]))

            # per-(v,half) output accumulators; created lazily
            out_ps = {}
            osb_tiles = {}

            # ---- pipeline --------------------------------------------
            # unit = (v, g).  pairs of consecutive groups share ACT batches.
            state = {}   # pair -> dict of tiles

            def head(pair):
                v, ga = pair
                ta2 = tap.tile([128, 2048], fp16, tag="ta",
                               name=f"ta_{v}_{ga}")
                for h, g in enumerate((ga, ga + 1)):
                    pz = pzp.tile([128, 1024], f32, tag="pz")
                    for half in range(2):
                        for j in range(4):
                            nc.tensor.matmul(
                                pz[:, half * VB:(half + 1) * VB],
                                lhsT=wt_t[:, j * 2048 + g * 256 + half * 128:
                                          j * 2048 + g * 256 + (half + 1) * 128],
                                rhs=xt_t[:, j * 1024 + v * VB:
                                         j * 1024 + (v + 1) * VB],
                                start=(j == 0), stop=(j == 3),
                            )
                    nc.vector.tensor_tensor(
                        out=ta2[:, h * 1024:(h + 1) * 1024]
                        .rearrange("p (two x) -> p two x", x=VB),
                        in0=pz[:].rearrange("p (two x) -> p two x", x=VB),
                        in1=bias_bc(g), op=ALU.add)
                state[pair] = dict(ta2=ta2)

            def tail(pair):
                v, ga = pair
                st = state.pop(pair)
                ta2 = st["ta2"]
                te2 = work.tile([128, 2048], fp16, tag="te",
                                name=f"te_{v}_{ga}")
                nc.scalar.activation(te2[:], ta2[:], AF.Exp)
                ta4 = ta2[:].rearrange("p (g two x) -> p g two x", g=2, x=VB)
                te4 = te2[:].rearrange("p (g two x) -> p g two x", g=2, x=VB)
                sp2 = work.tile([128, 1024], fp16, tag="sp",
                                name=f"sp_{v}_{ga}")
                sp4 = sp2[:].rearrange("p (g x) -> p g x", g=2)
                nc.scalar.activation(sp4, te4[:, :, 0, :], AF.Ln, bias=1.0)
                t1 = work.tile([128, 1024], fp16, tag="t1",
                               name=f"t1_{v}_{ga}")
                nc.gpsimd.tensor_scalar_add(
                    out=t1[:].rearrange("p (g x) -> p g x", g=2),
                    in0=te4[:, :, 1, :], scalar1=1.0)
                r5 = work.tile([128, 1024], f32, tag="r5",
                               name=f"r5_{v}_{ga}")
                nc.vector.reciprocal(r5[:], t1[:])

                pp = ppp.tile([128, 1024], f32, tag="pp")
                for h in range(2):
                    sl = slice(h * VB, (h + 1) * VB)
                    nc.tensor.matmul(pp[:, sl], lhsT=adir_ap,
                                     rhs=ta4[:, h, 0, :],
                                     start=True, stop=False)
                    nc.tensor.matmul(pp[:, sl], lhsT=apath_ap,
                                     rhs=sp2[:, sl],
                                     start=False, stop=True)
                ep = work.tile([128, 1024], fp16, tag="ep",
                               name=f"ep_{v}_{ga}")
                nc.scalar.activation(ep[:], pp[:], AF.Exp)
                epr = work.tile([128, 1024], fp16, tag="epr",
                                name=f"epr_{v}_{ga}")
                nc.gpsimd.tensor_tensor(out=epr[:], in0=ep[:], in1=r5[:],
                                        op=ALU.mult)

                half = "A" if ga < 4 else "B"
                if (v, half) not in out_ps:
                    out_ps[(v, half)] = pop.tile([128, VB], f32, tag="ops",
                                                 name=f"ops_{v}{half}")
                ops = out_ps[(v, half)]
                for h, g in enumerate((ga, ga + 1)):
                    sl = slice(h * VB, (h + 1) * VB)
                    nc.tensor.matmul(ops[:], lhsT=vm_t[:, g * 256:g * 256 + 128],
                                     rhs=ep[:, sl],
                                     start=(g % 4 == 0), stop=False)
                    nc.tensor.matmul(ops[:],
                                     lhsT=vm_t[:, g * 256 + 128:g * 256 + 256],
                                     rhs=epr[:, sl],
                                     start=False, stop=(g % 4 == 3))
                if ga + 1 in (3, 7):
                    osb = work.tile([128, VB], f32, tag=f"osb{half}",
                                    name=f"osb_{v}{half}")
                    nc.vector.tensor_copy(out=osb[:], in_=ops[:])
                    eng = nc.gpsimd if half == "A" else nc.sync
                    eng.dma_start(out=outs[(v, half)][:], in_=osb[0:100, :])
                    del out_ps[(v, half)]

            pairs = [(v, ga) for v in range(2) for ga in (0, 2, 4, 6)]
            pending = []
            for p in pairs:
                head(p)
                if pending and len(pending) >= 2:
                    tail(pending.pop(0))
                pending.append(p)
            while pending:
                tail(pending.pop(0))

    nc.finalize()
    return nc


def _get_nc():
    if "nc" not in _NC_CACHE:
        _NC_CACHE["nc"] = _build_bass()
    return _NC_CACHE["nc"]


def _prep_inputs(x, split_weights, split_bias, leaf_logits, tree_weights):
    x = np.asarray(x, np.float32)
    split_weights = np.asarray(split_weights, np.float32)
    split_bias = np.asarray(split_bias, np.float32)
    leaf_logits = np.asarray(leaf_logits, np.float32)
    tree_weights = np.asarray(tree_weights, np.float32)

    w_soft = np.exp(tree_weights - tree_weights.max())
    w_soft = w_soft / w_soft.sum()

    # leaf distributions scaled by 2*w_t (host softmax)
    ll = leaf_logits - leaf_logits.max(axis=-1, keepdims=True)
    ev = np.exp(ll)
    dist = ev / ev.sum(axis=-1, keepdims=True)          # [T, L, C]
    vt = 2.0 * w_soft[:, None, None] * dist             # [T, L, C]

    in_maps = []
    for tg in range(TG):
        trees = np.arange(tg * TL, (tg + 1) * TL)
        # weights: wt_np[j, g, p, c]
        wt_np = np.zeros((4, NG, 128, 256), np.float32)
        bias_np = np.zeros((128, 32), np.float32)
        vm_np = np.zeros((128, NG, 2, 128), np.float32)
        for tl, t in enumerate(trees):
            g, r = tl // 4, tl % 4
            rows = slice(32 * r, 32 * r + 32)
            W = split_weights[t]                        # [N, D]
            bs = split_bias[t]                          # [N]
            # shallow nodes 0..30 (+pad), deep nodes 31..62
            wsh = np.zeros((32, D), np.float32)
            wsh[:31] = W[0:31]
            wdp = W[31:63]                              # [32, D]
            for j in range(4):
                dsl = slice(j * 128, (j + 1) * 128)
                wt_np[j, g, :, 32 * r:32 * r + 32] = wsh[:, dsl].T
                wt_np[j, g, :, 128 + 32 * r:128 + 32 * r + 32] = wdp[:, dsl].T
            bias_np[rows, 2 * g] = np.concatenate([bs[0:31], [0.0]])
            bias_np[rows, 2 * g + 1] = bs[31:63]
            # V_A = V_odd, V_B = V_even - V_odd  (parents j = 0..31)
            v_even = vt[t, 0::2, :]                     # [32, C]
            v_odd = vt[t, 1::2, :]
            vm_np[rows, g, 0, :C] = v_odd
            vm_np[rows, g, 1, :C] = v_even - v_odd
        wt16 = np.ascontiguousarray(
            wt_np.reshape(4096, 256).astype(np.float16))
        vm16 = np.ascontiguousarray(
            vm_np.reshape(128, 2048).astype(np.float16))
        shared = dict(wt=wt16, consts=bias_np.copy(), amat=_AMAT, vmat=vm16)
        for bg in range(BG):
            xs = x[bg * BSL:(bg + 1) * BSL, :]          # [1024, 512]
            xt16 = np.ascontiguousarray(
                xs.T.reshape(4, 128, BSL).transpose(1, 0, 2)
                .reshape(128, 4096).astype(np.float16))
            in_maps.append(dict(xt=xt16, **shared))
    return in_maps


def kernel(x, split_weights, split_bias, leaf_logits, tree_weights):
    from concourse.bass_utils import run_bass_kernel_spmd

    in_maps = _prep_inputs(x, split_weights, split_bias, leaf_logits,
                           tree_weights)
    nc = _get_nc()
    res = run_bass_kernel_spmd(nc, in_maps, core_ids=list(range(NCORES)))
    out = np.zeros((B, C), np.float32)
    for tg in range(TG):
        for bg in range(BG):
            r = res.results[tg * BG + bg]
            for v in range(2):
                part = (r[f"oA{v}"] + r[f"oB{v}"]).T      # [512, 100]
                rows = slice(bg * BSL + v * VB, bg * BSL + (v + 1) * VB)
                out[rows] += part
    return np.ascontiguousarray(out)


# revision 6
# speedup vs baseline: 1.2708x; 1.2708x over previous
"""Trainium2 Bass kernel for the soft-decision-tree ensemble classifier (V4).

Restructure vs V3b: the deepest tree level is factored out of the log-domain
path matmul.  For parent node j (level-5 node 31+j), its two leaves satisfy
  lp[2j]   = exp(cm_j) * r5_j,      r5 = 1/(1+e^{z5})
  lp[2j+1] = exp(cm_j) * (1-r5_j)
with cm the level-0..4 path log-prob.  Folding into the output matmul with
V_A = V_odd, V_B = V_even - V_odd gives  out += V_A^T ep + V_B^T (ep*r5).
This halves the ScalarE exp/ln work (only 31 of 63 nodes need softplus), and
halves the A-matmul contraction (32-wide blocks, 4 trees per 128-tile).
The leaf-distribution softmax is computed on the host and DMA'd as V_A/V_B.

Sharding: 2-way trees x 4-way batch (unchanged).  Per core: 32 trees in 8
groups of 4, batch 1024 in 2 blocks of 512.  All matmuls fp16, N=512.
PSUM: pz pairs (sh|dp, 2 banks) x2, pp pair x1 (2 banks), out x2 = 8 banks.
Warm-up matmuls on a memset tile run during the DMA/preamble head so the PE
HAM clock-gate is released before real matmuls start.
"""

import numpy as np

TREE_DEPTH = 6
T, N, D, C = 64, 63, 512, 100
L = 2**TREE_DEPTH
TG = 2
BG = 4
TL = T // TG               # 32 trees per core
NG = 8                     # tree groups of 4 per core
B = 4096
NCORES = 8
BSL = B // BG              # 1024 batch rows per core
VB = 512

N_WARM = 32

_NC_CACHE = {}


def _parent_paths():
    """For each level-0..4 parent path j: the shallow nodes visited and the
    direction bit at each, checking the level-5 node is 31+j."""
    paths = []
    for j in range(32):
        node = 0
        steps = []
        for k in range(5):
            bit = (j >> (4 - k)) & 1
            steps.append((node, bit))
            node = 2 * node + 1 + bit
        assert node == 31 + j
        paths.append(steps)
    return paths


def _pack_amat():
    """[128, 256] fp16: [:, :128] = A'dir, [:, 128:] = A'path, block-diagonal
    over 4 trees of (32 shallow nodes x 32 parents)."""
    adir = np.zeros((32, 32), np.float32)
    apath = np.zeros((32, 32), np.float32)
    for j, steps in enumerate(_parent_paths()):
        for node, bit in steps:
            if bit:
                adir[node, j] += 1.0
            apath[node, j] -= 1.0
    amat = np.zeros((128, 256), np.float16)
    for r in range(4):
        sl = slice(32 * r, 32 * r + 32)
        amat[sl, sl] = adir
        amat[sl, 128 + 32 * r:128 + 32 * r + 32] = apath
    return amat


_AMAT = _pack_amat()


def _build_bass():
    import concourse.bacc as bacc
    import concourse.mybir as mybir
    import concourse.tile as tile
    from concourse.hw_specs import get_activation_tables

    dt = mybir.dt
    f32 = dt.float32
    f32r = dt.float32r
    fp16 = dt.float16
    AF = mybir.ActivationFunctionType
    ALU = mybir.AluOpType

    nc = bacc.Bacc("TRN2", target_bir_lowering=False, debug=False,
                   num_devices=NCORES)

    table_id = next(i for i, (_, funcs) in
                    enumerate(get_activation_tables("gen3").items())
                    if AF.Exp in funcs and AF.Ln in funcs)
    nc.scalar.add_instruction(mybir.InstLoadActFuncSet(
        name=f"I-{nc.next_id()}", ins=[], outs=[], act_func_set_id=table_id))

    # ---- DRAM tensors ------------------------------------------------
    xt = nc.dram_tensor("xt", [128, 4096], fp16, kind="ExternalInput").ap()
    wt = nc.dram_tensor("wt", [4096, 256], fp16, kind="ExternalInput").ap()
    consts = nc.dram_tensor("consts", [128, 32], f32r,
                            kind="ExternalInput").ap()
    amat = nc.dram_tensor("amat", [128, 256], fp16, kind="ExternalInput").ap()
    vmat = nc.dram_tensor("vmat", [128, 2048], fp16,
                          kind="ExternalInput").ap()
    outs = {}
    for v in range(2):
        for h in ("A", "B"):
            nm = f"o{h}{v}"
            outs[(v, h)] = nc.dram_tensor(nm, [100, VB], f32,
                                          kind="ExternalOutput").ap()

    with tile.TileContext(nc) as tc:
        with (
            tc.tile_pool(name="big", bufs=1) as bigp,
            tc.tile_pool(name="const", bufs=1) as constp,
            tc.tile_pool(name="ta", bufs=3) as tap,
            tc.tile_pool(name="work", bufs=2) as work,
            tc.tile_pool(name="pz", bufs=2, space="PSUM") as pzp,
            tc.tile_pool(name="pp", bufs=1, space="PSUM") as ppp,
            tc.tile_pool(name="po", bufs=2, space="PSUM") as pop,
        ):
            wt_t = bigp.tile([128, 8192], fp16, tag="wt")
            xt_t = bigp.tile([128, 4096], fp16, tag="xt")
            vm_t = bigp.tile([128, 2048], fp16, tag="vm")
            consts_t = constp.tile([128, 32], f32r, tag="consts")
            amat_t = constp.tile([128, 256], fp16, tag="amat")
            warm_t = constp.tile([128, 128], fp16, tag="warm")

            # ---- warm-up: memset tile + back-to-back matmuls ---------
            nc.gpsimd.memset(warm_t[:], 0.0)
            warm_ps = pop.tile([128, VB], f32, tag="ops", name="warm_ps")
            for _ in range(N_WARM):
                nc.tensor.matmul(warm_ps[:, 0:128], lhsT=warm_t[:],
                                 rhs=warm_t[:], start=True, stop=True)

            # ---- DMA plan --------------------------------------------
            k = 0

            def dma(out_, in_):
                nonlocal k
                eng = nc.gpsimd if k % 2 == 0 else nc.sync
                eng.dma_start(out=out_, in_=in_)
                k += 1

            def wt_piece(j, g0, g1):
                ng = g1 - g0
                dma(wt_t[:, j * 2048 + g0 * 256:j * 2048 + g1 * 256]
                    .rearrange("p (g c) -> p g c", c=256),
                    wt[j * 1024 + g0 * 128:j * 1024 + g1 * 128, :]
                    .rearrange("(g p) c -> p g c", p=128))

            def xt_piece(j, v):
                c0 = j * 1024 + v * VB
                dma(xt_t[:, c0:c0 + VB], xt[:, c0:c0 + VB])

            dma(amat_t[:], amat[:])
            dma(consts_t[:], consts[:])
            xt_piece(0, 0)
            wt_piece(0, 0, 2)
            wt_piece(1, 0, 2)
            xt_piece(1, 0)
            xt_piece(2, 0)
            wt_piece(2, 0, 2)
            wt_piece(3, 0, 2)
            xt_piece(3, 0)
            # bulk: rest of weights, vmat, x v1
            dma(vm_t[:, 0:512], vmat[:, 0:512])
            wt_piece(0, 2, 8)
            wt_piece(1, 2, 8)
            dma(vm_t[:, 512:1024], vmat[:, 512:1024])
            wt_piece(2, 2, 8)
            wt_piece(3, 2, 8)
            dma(vm_t[:, 1024:2048], vmat[:, 1024:2048])
            for j in range(4):
                xt_piece(j, 1)

            adir_ap = amat_t[:, 0:128]
            apath_ap = amat_t[:, 128:256]

            def bias_ap(c):
                return consts_t[:, c:c + 1].bitcast(f32)

            # per-(v,half) output accumulators; created lazily
            out_ps = {}
            osb_tiles = {}

            # ---- pipeline --------------------------------------------
            # unit = (v, g).  pairs of consecutive groups share ACT batches.
            state = {}   # pair -> dict of tiles

            def head(pair):
                v, ga = pair
                # ta2 layout: [sh_g | sh_g1 | dp_g | dp_g1]
                ta2 = tap.tile([128, 2048], fp16, tag="ta",
                               name=f"ta_{v}_{ga}")
                for h, g in enumerate((ga, ga + 1)):
                    pz = pzp.tile([128, 1024], f32, tag="pz")
                    for half in range(2):
                        for j in range(4):
                            nc.tensor.matmul(
                                pz[:, half * VB:(half + 1) * VB],
                                lhsT=wt_t[:, j * 2048 + g * 256 + half * 128:
                                          j * 2048 + g * 256 + (half + 1) * 128],
                                rhs=xt_t[:, j * 1024 + v * VB:
                                         j * 1024 + (v + 1) * VB],
                                start=(j == 0), stop=(j == 3),
                            )
                    nc.vector.tensor_scalar_add(
                        out=ta2[:, h * VB:(h + 1) * VB],
                        in0=pz[:, 0:VB], scalar1=bias_ap(2 * g))
                    nc.vector.tensor_scalar_add(
                        out=ta2[:, 1024 + h * VB:1024 + (h + 1) * VB],
                        in0=pz[:, VB:2 * VB], scalar1=bias_ap(2 * g + 1))
                te2 = work.tile([128, 2048], f32, tag="te",
                                name=f"te_{v}_{ga}")
                nc.scalar.activation(te2[:], ta2[:], AF.Exp)
                sp2 = work.tile([128, 1024], fp16, tag="sp",
                                name=f"sp_{v}_{ga}")
                nc.scalar.activation(sp2[:], te2[:, 0:1024], AF.Ln, bias=1.0)
                t1 = work.tile([128, 1024], f32, tag="t1",
                               name=f"t1_{v}_{ga}")
                nc.gpsimd.tensor_scalar_add(out=t1[:], in0=te2[:, 1024:2048],
                                            scalar1=1.0)
                r5 = work.tile([128, 1024], f32, tag="r5",
                               name=f"r5_{v}_{ga}")
                nc.vector.reciprocal_approx_fast(out=r5[:], in_=t1[:])
                state[pair] = dict(ta2=ta2, sp2=sp2, r5=r5)

            def tail(pair):
                v, ga = pair
                st = state.pop(pair)
                ta2, sp2, r5 = st["ta2"], st["sp2"], st["r5"]

                pp = ppp.tile([128, 1024], f32, tag="pp")
                for h in range(2):
                    sl = slice(h * VB, (h + 1) * VB)
                    nc.tensor.matmul(pp[:, sl], lhsT=adir_ap,
                                     rhs=ta2[:, sl],
                                     start=True, stop=False)
                    nc.tensor.matmul(pp[:, sl], lhsT=apath_ap,
                                     rhs=sp2[:, sl],
                                     start=False, stop=True)
                ep = work.tile([128, 1024], fp16, tag="ep",
                               name=f"ep_{v}_{ga}")
                nc.scalar.activation(ep[:], pp[:], AF.Exp)
                epr = work.tile([128, 1024], fp16, tag="epr",
                                name=f"epr_{v}_{ga}")
                nc.gpsimd.tensor_tensor(out=epr[:], in0=ep[:], in1=r5[:],
                                        op=ALU.mult)

                half = "A" if ga < 4 else "B"
                if (v, half) not in out_ps:
                    out_ps[(v, half)] = pop.tile([128, VB], f32, tag="ops",
                                                 name=f"ops_{v}{half}")
                ops = out_ps[(v, half)]
                for h, g in enumerate((ga, ga + 1)):
                    sl = slice(h * VB, (h + 1) * VB)
                    nc.tensor.matmul(ops[:], lhsT=vm_t[:, g * 256:g * 256 + 128],
                                     rhs=ep[:, sl],
                                     start=(g % 4 == 0), stop=False)
                    nc.tensor.matmul(ops[:],
                                     lhsT=vm_t[:, g * 256 + 128:g * 256 + 256],
                                     rhs=epr[:, sl],
                                     start=False, stop=(g % 4 == 3))
                if ga + 1 in (3, 7):
                    osb = work.tile([128, VB], f32, tag=f"osb{half}",
                                    name=f"osb_{v}{half}")
                    nc.vector.tensor_copy(out=osb[:], in_=ops[:])
                    eng = nc.gpsimd if half == "A" else nc.sync
                    eng.dma_start(out=outs[(v, half)][:], in_=osb[0:100, :])
                    del out_ps[(v, half)]

            pairs = [(v, ga) for v in range(2) for ga in (0, 2, 4, 6)]
            pending = []
            for p in pairs:
                head(p)
                if pending:
                    tail(pending.pop(0))
                pending.append(p)
            while pending:
                tail(pending.pop(0))

    nc.finalize()
    return nc


def _get_nc():
    if "nc" not in _NC_CACHE:
        _NC_CACHE["nc"] = _build_bass()
    return _NC_CACHE["nc"]


def _prep_inputs(x, split_weights, split_bias, leaf_logits, tree_weights):
    x = np.asarray(x, np.float32)
    split_weights = np.asarray(split_weights, np.float32)
    split_bias = np.asarray(split_bias, np.float32)
    leaf_logits = np.asarray(leaf_logits, np.float32)
    tree_weights = np.asarray(tree_weights, np.float32)

    w_soft = np.exp(tree_weights - tree_weights.max())
    w_soft = w_soft / w_soft.sum()

    # leaf distributions scaled by 2*w_t (host softmax)
    ll = leaf_logits - leaf_logits.max(axis=-1, keepdims=True)
    ev = np.exp(ll)
    dist = ev / ev.sum(axis=-1, keepdims=True)          # [T, L, C]
    vt = 2.0 * w_soft[:, None, None] * dist             # [T, L, C]

    in_maps = []
    for tg in range(TG):
        trees = np.arange(tg * TL, (tg + 1) * TL)
        # weights: wt_np[j, g, p, c]
        wt_np = np.zeros((4, NG, 128, 256), np.float32)
        bias_np = np.zeros((128, 32), np.float32)
        vm_np = np.zeros((128, NG, 2, 128), np.float32)
        for tl, t in enumerate(trees):
            g, r = tl // 4, tl % 4
            rows = slice(32 * r, 32 * r + 32)
            W = split_weights[t]                        # [N, D]
            bs = split_bias[t]                          # [N]
            # shallow nodes 0..30 (+pad), deep nodes 31..62
            wsh = np.zeros((32, D), np.float32)
            wsh[:31] = W[0:31]
            wdp = W[31:63]                              # [32, D]
            for j in range(4):
                dsl = slice(j * 128, (j + 1) * 128)
                wt_np[j, g, :, 32 * r:32 * r + 32] = wsh[:, dsl].T
                wt_np[j, g, :, 128 + 32 * r:128 + 32 * r + 32] = wdp[:, dsl].T
            bias_np[rows, 2 * g] = np.concatenate([bs[0:31], [0.0]])
            bias_np[rows, 2 * g + 1] = bs[31:63]
            # V_A = V_odd, V_B = V_even - V_odd  (parents j = 0..31)
            v_even = vt[t, 0::2, :]                     # [32, C]
            v_odd = vt[t, 1::2, :]
            vm_np[rows, g, 0, :C] = v_odd
            vm_np[rows, g, 1, :C] = v_even - v_odd
        wt16 = np.ascontiguousarray(
            wt_np.reshape(4096, 256).astype(np.float16))
        vm16 = np.ascontiguousarray(
            vm_np.reshape(128, 2048).astype(np.float16))
        shared = dict(wt=wt16, consts=bias_np.copy(), amat=_AMAT, vmat=vm16)
        for bg in range(BG):
            xs = x[bg * BSL:(bg + 1) * BSL, :]          # [1024, 512]
            xt16 = np.ascontiguousarray(
                xs.T.reshape(4, 128, BSL).transpose(1, 0, 2)
                .reshape(128, 4096).astype(np.float16))
            in_maps.append(dict(xt=xt16, **shared))
    return in_maps


def kernel(x, split_weights, split_bias, leaf_logits, tree_weights):
    from concourse.bass_utils import run_bass_kernel_spmd

    in_maps = _prep_inputs(x, split_weights, split_bias, leaf_logits,
                           tree_weights)
    nc = _get_nc()
    res = run_bass_kernel_spmd(nc, in_maps, core_ids=list(range(NCORES)))
    out = np.zeros((B, C), np.float32)
    for tg in range(TG):
        for bg in range(BG):
            r = res.results[tg * BG + bg]
            for v in range(2):
                part = (r[f"oA{v}"] + r[f"oB{v}"]).T      # [512, 100]
                rows = slice(bg * BSL + v * VB, bg * BSL + (v + 1) * VB)
                out[rows] += part
    return np.ascontiguousarray(out)


# revision 11
# speedup vs baseline: 2.4191x; 1.9035x over previous
"""Trainium2 Bass kernel for the soft-decision-tree ensemble classifier (V4).

Restructure vs V3b: the deepest tree level is factored out of the log-domain
path matmul.  For parent node j (level-5 node 31+j), its two leaves satisfy
  lp[2j]   = exp(cm_j) * r5_j,      r5 = 1/(1+e^{z5})
  lp[2j+1] = exp(cm_j) * (1-r5_j)
with cm the level-0..4 path log-prob.  Folding into the output matmul with
V_A = V_odd, V_B = V_even - V_odd gives  out += V_A^T ep + V_B^T (ep*r5).
This halves the ScalarE exp/ln work (only 31 of 63 nodes need softplus), and
halves the A-matmul contraction (32-wide blocks, 4 trees per 128-tile).
The leaf-distribution softmax is computed on the host and DMA'd as V_A/V_B.

Sharding: 2-way trees x 4-way batch (unchanged).  Per core: 32 trees in 8
groups of 4, batch 1024 in 2 blocks of 512.  All matmuls fp16, N=512.
PSUM: pz pairs (sh|dp, 2 banks) x2, pp pair x1 (2 banks), out x2 = 8 banks.
Warm-up matmuls on a memset tile run during the DMA/preamble head so the PE
HAM clock-gate is released before real matmuls start.
"""

import numpy as np

TREE_DEPTH = 6
T, N, D, C = 64, 63, 512, 100
L = 2**TREE_DEPTH
TG = 2
BG = 4
TL = T // TG               # 32 trees per core
NG = 8                     # tree groups of 4 per core
B = 4096
NCORES = 8
BSL = B // BG              # 1024 batch rows per core
VB = 512

N_WARM = 32

_NC_CACHE = {}


def _parent_paths():
    """For each level-0..4 parent path j: the shallow nodes visited and the
    direction bit at each, checking the level-5 node is 31+j."""
    paths = []
    for j in range(32):
        node = 0
        steps = []
        for k in range(5):
            bit = (j >> (4 - k)) & 1
            steps.append((node, bit))
            node = 2 * node + 1 + bit
        assert node == 31 + j
        paths.append(steps)
    return paths


def _pack_amat():
    """[128, 256] fp16: [:, :128] = A'dir, [:, 128:] = A'path, block-diagonal
    over 4 trees of (32 shallow nodes x 32 parents)."""
    adir = np.zeros((32, 32), np.float32)
    apath = np.zeros((32, 32), np.float32)
    for j, steps in enumerate(_parent_paths()):
        for node, bit in steps:
            if bit:
                adir[node, j] += 1.0
            apath[node, j] -= 1.0
    amat = np.zeros((128, 256), np.float16)
    for r in range(4):
        sl = slice(32 * r, 32 * r + 32)
        amat[sl, sl] = adir
        amat[sl, 128 + 32 * r:128 + 32 * r + 32] = apath
    return amat


_AMAT = _pack_amat()


def _build_bass():
    import concourse.bacc as bacc
    import concourse.mybir as mybir
    import concourse.tile as tile
    from concourse.hw_specs import get_activation_tables

    dt = mybir.dt
    f32 = dt.float32
    f32r = dt.float32r
    fp16 = dt.float16
    AF = mybir.ActivationFunctionType
    ALU = mybir.AluOpType

    nc = bacc.Bacc("TRN2", target_bir_lowering=False, debug=False,
                   num_devices=NCORES)

    table_id = next(i for i, (_, funcs) in
                    enumerate(get_activation_tables("gen3").items())
                    if AF.Exp in funcs and AF.Ln in funcs)
    nc.scalar.add_instruction(mybir.InstLoadActFuncSet(
        name=f"I-{nc.next_id()}", ins=[], outs=[], act_func_set_id=table_id))

    # ---- DRAM tensors ------------------------------------------------
    xt = nc.dram_tensor("xt", [128, 4096], fp16, kind="ExternalInput").ap()
    wt = nc.dram_tensor("wt", [4096, 256], fp16, kind="ExternalInput").ap()
    consts = nc.dram_tensor("consts", [128, 32], f32r,
                            kind="ExternalInput").ap()
    amat = nc.dram_tensor("amat", [128, 256], fp16, kind="ExternalInput").ap()
    vmat = nc.dram_tensor("vmat", [128, 2048], fp16,
                          kind="ExternalInput").ap()
    outs = {}
    for v in range(2):
        for h in ("A", "B"):
            nm = f"o{h}{v}"
            outs[(v, h)] = nc.dram_tensor(nm, [100, VB], f32,
                                          kind="ExternalOutput").ap()

    with tile.TileContext(nc) as tc:
        with (
            tc.tile_pool(name="big", bufs=1) as bigp,
            tc.tile_pool(name="const", bufs=1) as constp,
            tc.tile_pool(name="ta", bufs=3) as tap,
            tc.tile_pool(name="work", bufs=3) as work,
            tc.tile_pool(name="pz", bufs=2, space="PSUM") as pzp,
            tc.tile_pool(name="pp", bufs=1, space="PSUM") as ppp,
            tc.tile_pool(name="po", bufs=2, space="PSUM") as pop,
        ):
            wt_t = bigp.tile([128, 8192], fp16, tag="wt")
            xt_t = bigp.tile([128, 4096], fp16, tag="xt")
            vm_t = bigp.tile([128, 2048], fp16, tag="vm")
            consts_t = constp.tile([128, 32], f32r, tag="consts")
            amat_t = constp.tile([128, 256], fp16, tag="amat")
            warm_t = constp.tile([128, 128], fp16, tag="warm")

            # ---- warm-up: memset tile + back-to-back matmuls ---------
            nc.gpsimd.memset(warm_t[:], 0.0)
            warm_ps = pop.tile([128, VB], f32, tag="ops", name="warm_ps")
            for _ in range(N_WARM):
                nc.tensor.matmul(warm_ps[:, 0:128], lhsT=warm_t[:],
                                 rhs=warm_t[:], start=True, stop=True)

            # ---- DMA plan --------------------------------------------
            k = 0

            def dma(out_, in_):
                nonlocal k
                eng = nc.gpsimd if k % 2 == 0 else nc.sync
                eng.dma_start(out=out_, in_=in_)
                k += 1

            def wt_piece(j, g0, g1):
                ng = g1 - g0
                dma(wt_t[:, j * 2048 + g0 * 256:j * 2048 + g1 * 256]
                    .rearrange("p (g c) -> p g c", c=256),
                    wt[j * 1024 + g0 * 128:j * 1024 + g1 * 128, :]
                    .rearrange("(g p) c -> p g c", p=128))

            def xt_piece(j, v):
                c0 = j * 1024 + v * VB
                dma(xt_t[:, c0:c0 + VB], xt[:, c0:c0 + VB])

            dma(amat_t[:], amat[:])
            dma(consts_t[:], consts[:])
            xt_piece(0, 0)
            wt_piece(0, 0, 2)
            wt_piece(1, 0, 2)
            xt_piece(1, 0)
            xt_piece(2, 0)
            wt_piece(2, 0, 2)
            wt_piece(3, 0, 2)
            xt_piece(3, 0)
            # bulk: rest of weights, vmat, x v1
            dma(vm_t[:, 0:512], vmat[:, 0:512])
            wt_piece(0, 2, 8)
            wt_piece(1, 2, 8)
            dma(vm_t[:, 512:1024], vmat[:, 512:1024])
            wt_piece(2, 2, 8)
            wt_piece(3, 2, 8)
            dma(vm_t[:, 1024:2048], vmat[:, 1024:2048])
            for j in range(4):
                xt_piece(j, 1)

            adir_ap = amat_t[:, 0:128]
            apath_ap = amat_t[:, 128:256]

            def bias_ap(c):
                return consts_t[:, c:c + 1].bitcast(f32)

            ones_bc = (consts_t[:, 16:17].bitcast(f32)
                       .broadcast_to([128, 1024]))

            # per-(v,half) output accumulators; created lazily
            out_ps = {}
            osb_tiles = {}

            # ---- pipeline --------------------------------------------
            # unit = (v, g).  pairs of consecutive groups share ACT batches.
            state = {}   # pair -> dict of tiles

            def head(pair):
                v, ga = pair
                # ta2 layout: [sh_g | sh_g1 | dp_g | dp_g1]
                ta2 = tap.tile([128, 2048], fp16, tag="ta",
                               name=f"ta_{v}_{ga}")
                for h, g in enumerate((ga, ga + 1)):
                    pz = pzp.tile([128, 1024], f32, tag="pz")
                    for half in range(2):
                        for j in range(4):
                            nc.tensor.matmul(
                                pz[:, half * VB:(half + 1) * VB],
                                lhsT=wt_t[:, j * 2048 + g * 256 + half * 128:
                                          j * 2048 + g * 256 + (half + 1) * 128],
                                rhs=xt_t[:, j * 1024 + v * VB:
                                         j * 1024 + (v + 1) * VB],
                                start=(j == 0), stop=(j == 3),
                            )
                    nc.vector.tensor_scalar_add(
                        out=ta2[:, h * VB:(h + 1) * VB],
                        in0=pz[:, 0:VB], scalar1=bias_ap(2 * g))
                    nc.vector.tensor_scalar_add(
                        out=ta2[:, 1024 + h * VB:1024 + (h + 1) * VB],
                        in0=pz[:, VB:2 * VB], scalar1=bias_ap(2 * g + 1))
                te2 = work.tile([128, 2048], f32, tag="te",
                                name=f"te_{v}_{ga}")
                nc.scalar.activation(te2[:], ta2[:], AF.Exp)
                sp2 = work.tile([128, 1024], fp16, tag="sp",
                                name=f"sp_{v}_{ga}")
                nc.scalar.activation(sp2[:], te2[:, 0:1024], AF.Ln, bias=1.0)
                t1 = work.tile([128, 1024], f32, tag="t1",
                               name=f"t1_{v}_{ga}")
                nc.gpsimd.tensor_tensor(out=t1[:], in0=te2[:, 1024:2048],
                                        in1=ones_bc, op=ALU.add)
                r5 = work.tile([128, 1024], f32, tag="r5",
                               name=f"r5_{v}_{ga}")
                nc.vector.reciprocal_approx_fast(out=r5[:], in_=t1[:])
                state[pair] = dict(ta2=ta2, sp2=sp2, r5=r5)

            def tail(pair):
                v, ga = pair
                st = state.pop(pair)
                ta2, sp2, r5 = st["ta2"], st["sp2"], st["r5"]

                pp = ppp.tile([128, 1024], f32, tag="pp")
                for h in range(2):
                    sl = slice(h * VB, (h + 1) * VB)
                    nc.tensor.matmul(pp[:, sl], lhsT=adir_ap,
                                     rhs=ta2[:, sl],
                                     start=True, stop=False)
                    nc.tensor.matmul(pp[:, sl], lhsT=apath_ap,
                                     rhs=sp2[:, sl],
                                     start=False, stop=True)
                ep = work.tile([128, 1024], fp16, tag="ep",
                               name=f"ep_{v}_{ga}")
                nc.scalar.activation(ep[:], pp[:], AF.Exp)
                epr = work.tile([128, 1024], fp16, tag="epr",
                                name=f"epr_{v}_{ga}")
                nc.gpsimd.tensor_tensor(out=epr[:], in0=ep[:], in1=r5[:],
                                        op=ALU.mult)

                half = "A" if ga < 4 else "B"
                if (v, half) not in out_ps:
                    out_ps[(v, half)] = pop.tile([128, VB], f32, tag="ops",
                                                 name=f"ops_{v}{half}")
                ops = out_ps[(v, half)]
                for h, g in enumerate((ga, ga + 1)):
                    sl = slice(h * VB, (h + 1) * VB)
                    nc.tensor.matmul(ops[:], lhsT=vm_t[:, g * 256:g * 256 + 128],
                                     rhs=ep[:, sl],
                                     start=(g % 4 == 0), stop=False)
                    nc.tensor.matmul(ops[:],
                                     lhsT=vm_t[:, g * 256 + 128:g * 256 + 256],
                                     rhs=epr[:, sl],
                                     start=False, stop=(g % 4 == 3))
                if ga + 1 in (3, 7):
                    osb = work.tile([128, VB], f32, tag=f"osb{half}",
                                    name=f"osb_{v}{half}")
                    nc.vector.tensor_copy(out=osb[:], in_=ops[:])
                    eng = nc.gpsimd if half == "A" else nc.sync
                    eng.dma_start(out=outs[(v, half)][:], in_=osb[0:100, :])
                    del out_ps[(v, half)]

            pairs = [(v, ga) for v in range(2) for ga in (0, 2, 4, 6)]
            pending = []
            for p in pairs:
                head(p)
                if len(pending) >= 2:
                    tail(pending.pop(0))
                pending.append(p)
            while pending:
                tail(pending.pop(0))

    nc.finalize()
    return nc


def _get_nc():
    if "nc" not in _NC_CACHE:
        _NC_CACHE["nc"] = _build_bass()
    return _NC_CACHE["nc"]


def _prep_inputs(x, split_weights, split_bias, leaf_logits, tree_weights):
    x = np.asarray(x, np.float32)
    split_weights = np.asarray(split_weights, np.float32)
    split_bias = np.asarray(split_bias, np.float32)
    leaf_logits = np.asarray(leaf_logits, np.float32)
    tree_weights = np.asarray(tree_weights, np.float32)

    w_soft = np.exp(tree_weights - tree_weights.max())
    w_soft = w_soft / w_soft.sum()

    # leaf distributions scaled by 2*w_t (host softmax)
    ll = leaf_logits - leaf_logits.max(axis=-1, keepdims=True)
    ev = np.exp(ll)
    dist = ev / ev.sum(axis=-1, keepdims=True)          # [T, L, C]
    vt = 2.0 * w_soft[:, None, None] * dist             # [T, L, C]

    in_maps = []
    for tg in range(TG):
        trees = np.arange(tg * TL, (tg + 1) * TL)
        # weights: wt_np[j, g, p, c]
        wt_np = np.zeros((4, NG, 128, 256), np.float32)
        bias_np = np.zeros((128, 32), np.float32)
        vm_np = np.zeros((128, NG, 2, 128), np.float32)
        for tl, t in enumerate(trees):
            g, r = tl // 4, tl % 4
            rows = slice(32 * r, 32 * r + 32)
            W = split_weights[t]                        # [N, D]
            bs = split_bias[t]                          # [N]
            # shallow nodes 0..30 (+pad), deep nodes 31..62
            wsh = np.zeros((32, D), np.float32)
            wsh[:31] = W[0:31]
            wdp = W[31:63]                              # [32, D]
            for j in range(4):
                dsl = slice(j * 128, (j + 1) * 128)
                wt_np[j, g, :, 32 * r:32 * r + 32] = wsh[:, dsl].T
                wt_np[j, g, :, 128 + 32 * r:128 + 32 * r + 32] = wdp[:, dsl].T
            bias_np[rows, 2 * g] = np.concatenate([bs[0:31], [0.0]])
            bias_np[rows, 2 * g + 1] = bs[31:63]
            # V_A = V_odd, V_B = V_even - V_odd  (parents j = 0..31)
            v_even = vt[t, 0::2, :]                     # [32, C]
            v_odd = vt[t, 1::2, :]
            vm_np[rows, g, 0, :C] = v_odd
            vm_np[rows, g, 1, :C] = v_even - v_odd
        wt16 = np.ascontiguousarray(
            wt_np.reshape(4096, 256).astype(np.float16))
        vm16 = np.ascontiguousarray(
            vm_np.reshape(128, 2048).astype(np.float16))
        bias_np[:, 16] = 1.0
        shared = dict(wt=wt16, consts=bias_np.copy(), amat=_AMAT, vmat=vm16)
        for bg in range(BG):
            xs = x[bg * BSL:(bg + 1) * BSL, :]          # [1024, 512]
            xt16 = np.ascontiguousarray(
                xs.T.reshape(4, 128, BSL).transpose(1, 0, 2)
                .reshape(128, 4096).astype(np.float16))
            in_maps.append(dict(xt=xt16, **shared))
    return in_maps


def kernel(x, split_weights, split_bias, leaf_logits, tree_weights):
    from concourse.bass_utils import run_bass_kernel_spmd

    in_maps = _prep_inputs(x, split_weights, split_bias, leaf_logits,
                           tree_weights)
    nc = _get_nc()
    res = run_bass_kernel_spmd(nc, in_maps, core_ids=list(range(NCORES)))
    out = np.zeros((B, C), np.float32)
    for tg in range(TG):
        for bg in range(BG):
            r = res.results[tg * BG + bg]
            for v in range(2):
                part = (r[f"oA{v}"] + r[f"oB{v}"]).T      # [512, 100]
                rows = slice(bg * BSL + v * VB, bg * BSL + (v + 1) * VB)
                out[rows] += part
    return np.ascontiguousarray(out)


# revision 12
# speedup vs baseline: 2.8072x; 1.1605x over previous
"""Trainium2 Bass kernel for the soft-decision-tree ensemble classifier (V4).

Restructure vs V3b: the deepest tree level is factored out of the log-domain
path matmul.  For parent node j (level-5 node 31+j), its two leaves satisfy
  lp[2j]   = exp(cm_j) * r5_j,      r5 = 1/(1+e^{z5})
  lp[2j+1] = exp(cm_j) * (1-r5_j)
with cm the level-0..4 path log-prob.  Folding into the output matmul with
V_A = V_odd, V_B = V_even - V_odd gives  out += V_A^T ep + V_B^T (ep*r5).
This halves the ScalarE exp/ln work (only 31 of 63 nodes need softplus), and
halves the A-matmul contraction (32-wide blocks, 4 trees per 128-tile).
The leaf-distribution softmax is computed on the host and DMA'd as V_A/V_B.

Sharding: 2-way trees x 4-way batch (unchanged).  Per core: 32 trees in 8
groups of 4, batch 1024 in 2 blocks of 512.  All matmuls fp16, N=512.
PSUM: pz pairs (sh|dp, 2 banks) x2, pp pair x1 (2 banks), out x2 = 8 banks.
Warm-up matmuls on a memset tile run during the DMA/preamble head so the PE
HAM clock-gate is released before real matmuls start.
"""

import numpy as np

TREE_DEPTH = 6
T, N, D, C = 64, 63, 512, 100
L = 2**TREE_DEPTH
TG = 2
BG = 4
TL = T // TG               # 32 trees per core
NG = 8                     # tree groups of 4 per core
B = 4096
NCORES = 8
BSL = B // BG              # 1024 batch rows per core
VB = 512

N_WARM = 32

_NC_CACHE = {}


def _parent_paths():
    """For each level-0..4 parent path j: the shallow nodes visited and the
    direction bit at each, checking the level-5 node is 31+j."""
    paths = []
    for j in range(32):
        node = 0
        steps = []
        for k in range(5):
            bit = (j >> (4 - k)) & 1
            steps.append((node, bit))
            node = 2 * node + 1 + bit
        assert node == 31 + j
        paths.append(steps)
    return paths


def _pack_amat():
    """[128, 256] fp16: [:, :128] = A'dir, [:, 128:] = A'path, block-diagonal
    over 4 trees of (32 shallow nodes x 32 parents)."""
    adir = np.zeros((32, 32), np.float32)
    apath = np.zeros((32, 32), np.float32)
    for j, steps in enumerate(_parent_paths()):
        for node, bit in steps:
            if bit:
                adir[node, j] += 1.0
            apath[node, j] -= 1.0
    amat = np.zeros((128, 256), np.float16)
    for r in range(4):
        sl = slice(32 * r, 32 * r + 32)
        amat[sl, sl] = adir
        amat[sl, 128 + 32 * r:128 + 32 * r + 32] = apath
    return amat


_AMAT = _pack_amat()


def _build_bass():
    import concourse.bacc as bacc
    import concourse.mybir as mybir
    import concourse.tile as tile
    from concourse.hw_specs import get_activation_tables

    dt = mybir.dt
    f32 = dt.float32
    f32r = dt.float32r
    fp16 = dt.float16
    AF = mybir.ActivationFunctionType
    ALU = mybir.AluOpType

    nc = bacc.Bacc("TRN2", target_bir_lowering=False, debug=False,
                   num_devices=NCORES)

    table_id = next(i for i, (_, funcs) in
                    enumerate(get_activation_tables("gen3").items())
                    if AF.Exp in funcs and AF.Ln in funcs)
    nc.scalar.add_instruction(mybir.InstLoadActFuncSet(
        name=f"I-{nc.next_id()}", ins=[], outs=[], act_func_set_id=table_id))

    # ---- DRAM tensors ------------------------------------------------
    xt = nc.dram_tensor("xt", [128, 4096], fp16, kind="ExternalInput").ap()
    wt = nc.dram_tensor("wt", [4096, 256], fp16, kind="ExternalInput").ap()
    consts = nc.dram_tensor("consts", [128, 32], f32r,
                            kind="ExternalInput").ap()
    amat = nc.dram_tensor("amat", [128, 256], fp16, kind="ExternalInput").ap()
    vmat = nc.dram_tensor("vmat", [128, 2048], fp16,
                          kind="ExternalInput").ap()
    outs = {}
    for v in range(2):
        for h in ("A", "B"):
            nm = f"o{h}{v}"
            outs[(v, h)] = nc.dram_tensor(nm, [100, VB], f32,
                                          kind="ExternalOutput").ap()

    with tile.TileContext(nc) as tc:
        with (
            tc.tile_pool(name="big", bufs=1) as bigp,
            tc.tile_pool(name="const", bufs=1) as constp,
            tc.tile_pool(name="ta", bufs=3) as tap,
            tc.tile_pool(name="work", bufs=3) as work,
            tc.tile_pool(name="pz", bufs=2, space="PSUM") as pzp,
            tc.tile_pool(name="pp", bufs=1, space="PSUM") as ppp,
            tc.tile_pool(name="po", bufs=2, space="PSUM") as pop,
        ):
            wt_t = bigp.tile([128, 8192], fp16, tag="wt")
            xt_t = bigp.tile([128, 4096], fp16, tag="xt")
            vm_t = bigp.tile([128, 2048], fp16, tag="vm")
            consts_t = constp.tile([128, 32], f32r, tag="consts")
            amat_t = constp.tile([128, 256], fp16, tag="amat")
            warm_t = constp.tile([128, 128], fp16, tag="warm")

            # ---- warm-up: memset tile + back-to-back matmuls ---------
            nc.gpsimd.memset(warm_t[:], 0.0)
            warm_ps = pop.tile([128, VB], f32, tag="ops", name="warm_ps")
            for _ in range(N_WARM):
                nc.tensor.matmul(warm_ps[:, 0:128], lhsT=warm_t[:],
                                 rhs=warm_t[:], start=True, stop=True)

            # ---- DMA plan --------------------------------------------
            k = 0

            def dma(out_, in_):
                nonlocal k
                eng = nc.gpsimd if k % 2 == 0 else nc.sync
                eng.dma_start(out=out_, in_=in_)
                k += 1

            def wt_piece(j, g0, g1):
                ng = g1 - g0
                dma(wt_t[:, j * 2048 + g0 * 256:j * 2048 + g1 * 256]
                    .rearrange("p (g c) -> p g c", c=256),
                    wt[j * 1024 + g0 * 128:j * 1024 + g1 * 128, :]
                    .rearrange("(g p) c -> p g c", p=128))

            def xt_piece(j, v):
                c0 = j * 1024 + v * VB
                dma(xt_t[:, c0:c0 + VB], xt[:, c0:c0 + VB])

            dma(amat_t[:], amat[:])
            dma(consts_t[:], consts[:])
            xt_piece(0, 0)
            wt_piece(0, 0, 2)
            wt_piece(1, 0, 2)
            xt_piece(1, 0)
            xt_piece(2, 0)
            wt_piece(2, 0, 2)
            wt_piece(3, 0, 2)
            xt_piece(3, 0)
            # bulk: rest of weights, vmat, x v1
            dma(vm_t[:, 0:512], vmat[:, 0:512])
            wt_piece(0, 2, 8)
            wt_piece(1, 2, 8)
            dma(vm_t[:, 512:1024], vmat[:, 512:1024])
            wt_piece(2, 2, 8)
            wt_piece(3, 2, 8)
            dma(vm_t[:, 1024:2048], vmat[:, 1024:2048])
            for j in range(4):
                xt_piece(j, 1)

            adir_ap = amat_t[:, 0:128]
            apath_ap = amat_t[:, 128:256]

            def bias_ap(c):
                return consts_t[:, c:c + 1].bitcast(f32)

            ones_bc = (consts_t[:, 16:17].bitcast(f32)
                       .broadcast_to([128, 1024]))

            # per-(v,half) output accumulators; created lazily
            out_ps = {}
            osb_tiles = {}

            # ---- pipeline --------------------------------------------
            # unit = (v, g).  pairs of consecutive groups share ACT batches.
            state = {}   # pair -> dict of tiles

            def head(pair):
                v, ga = pair
                # ta2 layout: [sh_g | sh_g1 | dp_g | dp_g1]
                ta2 = tap.tile([128, 2048], fp16, tag="ta",
                               name=f"ta_{v}_{ga}")
                pzs = []
                for h, g in enumerate((ga, ga + 1)):
                    pz = pzp.tile([128, 1024], f32, tag="pz")
                    pzs.append(pz)
                    for half in range(2):
                        for j in range(4):
                            nc.tensor.matmul(
                                pz[:, half * VB:(half + 1) * VB],
                                lhsT=wt_t[:, j * 2048 + g * 256 + half * 128:
                                          j * 2048 + g * 256 + (half + 1) * 128],
                                rhs=xt_t[:, j * 1024 + v * VB:
                                         j * 1024 + (v + 1) * VB],
                                start=(j == 0), stop=(j == 3),
                            )
                # shallow evacs first so sp's inputs are ready earliest
                for h, g in enumerate((ga, ga + 1)):
                    nc.vector.tensor_scalar_add(
                        out=ta2[:, h * VB:(h + 1) * VB],
                        in0=pzs[h][:, 0:VB], scalar1=bias_ap(2 * g))
                for h, g in enumerate((ga, ga + 1)):
                    nc.vector.tensor_scalar_add(
                        out=ta2[:, 1024 + h * VB:1024 + (h + 1) * VB],
                        in0=pzs[h][:, VB:2 * VB], scalar1=bias_ap(2 * g + 1))
                te2 = work.tile([128, 2048], f32, tag="te",
                                name=f"te_{v}_{ga}")
                nc.scalar.activation(te2[:, 0:1024], ta2[:, 0:1024], AF.Exp)
                sp2 = work.tile([128, 1024], fp16, tag="sp",
                                name=f"sp_{v}_{ga}")
                nc.scalar.activation(sp2[:], te2[:, 0:1024], AF.Ln, bias=1.0)
                nc.scalar.activation(te2[:, 1024:2048], ta2[:, 1024:2048],
                                     AF.Exp)
                state[pair] = dict(ta2=ta2, sp2=sp2, te2=te2)

            def tail_a(pair):
                v, ga = pair
                st = state[pair]
                ta2, sp2, te2 = st["ta2"], st["sp2"], st["te2"]

                pp = ppp.tile([128, 1024], f32, tag="pp")
                ep = work.tile([128, 1024], fp16, tag="ep",
                               name=f"ep_{v}_{ga}")
                for h in range(2):
                    sl = slice(h * VB, (h + 1) * VB)
                    nc.tensor.matmul(pp[:, sl], lhsT=adir_ap,
                                     rhs=ta2[:, sl],
                                     start=True, stop=False)
                    nc.tensor.matmul(pp[:, sl], lhsT=apath_ap,
                                     rhs=sp2[:, sl],
                                     start=False, stop=True)
                    nc.scalar.activation(ep[:, sl], pp[:, sl], AF.Exp)
                t1 = work.tile([128, 1024], f32, tag="t1",
                               name=f"t1_{v}_{ga}")
                nc.gpsimd.tensor_tensor(out=t1[:], in0=te2[:, 1024:2048],
                                        in1=ones_bc, op=ALU.add)
                r5 = work.tile([128, 1024], f32, tag="r5",
                               name=f"r5_{v}_{ga}")
                nc.vector.reciprocal_approx_fast(out=r5[:], in_=t1[:])
                st["ep"] = ep
                st["r5"] = r5

            def tail_b(pair):
                v, ga = pair
                st = state.pop(pair)
                ep, r5 = st["ep"], st["r5"]
                epr = work.tile([128, 1024], fp16, tag="epr",
                                name=f"epr_{v}_{ga}")
                for h in range(2):
                    sl = slice(h * VB, (h + 1) * VB)
                    nc.gpsimd.tensor_tensor(out=epr[:, sl], in0=ep[:, sl],
                                            in1=r5[:, sl], op=ALU.mult)

                half = "A" if ga < 4 else "B"
                if (v, half) not in out_ps:
                    out_ps[(v, half)] = pop.tile([128, VB], f32, tag="ops",
                                                 name=f"ops_{v}{half}")
                ops = out_ps[(v, half)]
                for h, g in enumerate((ga, ga + 1)):
                    sl = slice(h * VB, (h + 1) * VB)
                    nc.tensor.matmul(ops[:], lhsT=vm_t[:, g * 256:g * 256 + 128],
                                     rhs=ep[:, sl],
                                     start=(g % 4 == 0), stop=False)
                    nc.tensor.matmul(ops[:],
                                     lhsT=vm_t[:, g * 256 + 128:g * 256 + 256],
                                     rhs=epr[:, sl],
                                     start=False, stop=(g % 4 == 3))
                if ga + 1 in (3, 7):
                    osb = work.tile([128, VB], f32, tag=f"osb{half}",
                                    name=f"osb_{v}{half}")
                    nc.vector.tensor_copy(out=osb[:], in_=ops[:])
                    eng = nc.gpsimd if half == "A" else nc.sync
                    eng.dma_start(out=outs[(v, half)][:], in_=osb[0:100, :])
                    del out_ps[(v, half)]

            pairs = [(v, ga) for v in range(2) for ga in (0, 2, 4, 6)]
            for i, p in enumerate(pairs):
                head(p)
                if i >= 1:
                    tail_a(pairs[i - 1])
                if i >= 2:
                    tail_b(pairs[i - 2])
            tail_a(pairs[-1])
            tail_b(pairs[-2])
            tail_b(pairs[-1])

    nc.finalize()
    return nc


def _get_nc():
    if "nc" not in _NC_CACHE:
        _NC_CACHE["nc"] = _build_bass()
    return _NC_CACHE["nc"]


def _prep_inputs(x, split_weights, split_bias, leaf_logits, tree_weights):
    x = np.asarray(x, np.float32)
    split_weights = np.asarray(split_weights, np.float32)
    split_bias = np.asarray(split_bias, np.float32)
    leaf_logits = np.asarray(leaf_logits, np.float32)
    tree_weights = np.asarray(tree_weights, np.float32)

    w_soft = np.exp(tree_weights - tree_weights.max())
    w_soft = w_soft / w_soft.sum()

    # leaf distributions scaled by 2*w_t (host softmax)
    ll = leaf_logits - leaf_logits.max(axis=-1, keepdims=True)
    ev = np.exp(ll)
    dist = ev / ev.sum(axis=-1, keepdims=True)          # [T, L, C]
    vt = 2.0 * w_soft[:, None, None] * dist             # [T, L, C]

    in_maps = []
    for tg in range(TG):
        trees = np.arange(tg * TL, (tg + 1) * TL)
        # weights: wt_np[j, g, p, c]
        wt_np = np.zeros((4, NG, 128, 256), np.float32)
        bias_np = np.zeros((128, 32), np.float32)
        vm_np = np.zeros((128, NG, 2, 128), np.float32)
        for tl, t in enumerate(trees):
            g, r = tl // 4, tl % 4
            rows = slice(32 * r, 32 * r + 32)
            W = split_weights[t]                        # [N, D]
            bs = split_bias[t]                          # [N]
            # shallow nodes 0..30 (+pad), deep nodes 31..62
            wsh = np.zeros((32, D), np.float32)
            wsh[:31] = W[0:31]
            wdp = W[31:63]                              # [32, D]
            for j in range(4):
                dsl = slice(j * 128, (j + 1) * 128)
                wt_np[j, g, :, 32 * r:32 * r + 32] = wsh[:, dsl].T
                wt_np[j, g, :, 128 + 32 * r:128 + 32 * r + 32] = wdp[:, dsl].T
            bias_np[rows, 2 * g] = np.concatenate([bs[0:31], [0.0]])
            bias_np[rows, 2 * g + 1] = bs[31:63]
            # V_A = V_odd, V_B = V_even - V_odd  (parents j = 0..31)
            v_even = vt[t, 0::2, :]                     # [32, C]
            v_odd = vt[t, 1::2, :]
            vm_np[rows, g, 0, :C] = v_odd
            vm_np[rows, g, 1, :C] = v_even - v_odd
        wt16 = np.ascontiguousarray(
            wt_np.reshape(4096, 256).astype(np.float16))
        vm16 = np.ascontiguousarray(
            vm_np.reshape(128, 2048).astype(np.float16))
        bias_np[:, 16] = 1.0
        shared = dict(wt=wt16, consts=bias_np.copy(), amat=_AMAT, vmat=vm16)
        for bg in range(BG):
            xs = x[bg * BSL:(bg + 1) * BSL, :]          # [1024, 512]
            xt16 = np.ascontiguousarray(
                xs.T.reshape(4, 128, BSL).transpose(1, 0, 2)
                .reshape(128, 4096).astype(np.float16))
            in_maps.append(dict(xt=xt16, **shared))
    return in_maps


def kernel(x, split_weights, split_bias, leaf_logits, tree_weights):
    from concourse.bass_utils import run_bass_kernel_spmd

    in_maps = _prep_inputs(x, split_weights, split_bias, leaf_logits,
                           tree_weights)
    nc = _get_nc()
    res = run_bass_kernel_spmd(nc, in_maps, core_ids=list(range(NCORES)))
    out = np.zeros((B, C), np.float32)
    for tg in range(TG):
        for bg in range(BG):
            r = res.results[tg * BG + bg]
            for v in range(2):
                part = (r[f"oA{v}"] + r[f"oB{v}"]).T      # [512, 100]
                rows = slice(bg * BSL + v * VB, bg * BSL + (v + 1) * VB)
                out[rows] += part
    return np.ascontiguousarray(out)


# revision 16
# speedup vs baseline: 3.1733x; 1.1304x over previous
"""Trainium2 Bass kernel for the soft-decision-tree ensemble classifier (V4).

Restructure vs V3b: the deepest tree level is factored out of the log-domain
path matmul.  For parent node j (level-5 node 31+j), its two leaves satisfy
  lp[2j]   = exp(cm_j) * r5_j,      r5 = 1/(1+e^{z5})
  lp[2j+1] = exp(cm_j) * (1-r5_j)
with cm the level-0..4 path log-prob.  Folding into the output matmul with
V_A = V_odd, V_B = V_even - V_odd gives  out += V_A^T ep + V_B^T (ep*r5).
This halves the ScalarE exp/ln work (only 31 of 63 nodes need softplus), and
halves the A-matmul contraction (32-wide blocks, 4 trees per 128-tile).
The leaf-distribution softmax is computed on the host and DMA'd as V_A/V_B.

Sharding: 2-way trees x 4-way batch (unchanged).  Per core: 32 trees in 8
groups of 4, batch 1024 in 2 blocks of 512.  All matmuls fp16, N=512.
PSUM: pz pairs (sh|dp, 2 banks) x2, pp pair x1 (2 banks), out x2 = 8 banks.
Warm-up matmuls on a memset tile run during the DMA/preamble head so the PE
HAM clock-gate is released before real matmuls start.
"""

import numpy as np

TREE_DEPTH = 6
T, N, D, C = 64, 63, 512, 100
L = 2**TREE_DEPTH
TG = 2
BG = 4
TL = T // TG               # 32 trees per core
NG = 8                     # tree groups of 4 per core
B = 4096
NCORES = 8
BSL = B // BG              # 1024 batch rows per core
VB = 512

N_WARM = 48

_NC_CACHE = {}


def _parent_paths():
    """For each level-0..4 parent path j: the shallow nodes visited and the
    direction bit at each, checking the level-5 node is 31+j."""
    paths = []
    for j in range(32):
        node = 0
        steps = []
        for k in range(5):
            bit = (j >> (4 - k)) & 1
            steps.append((node, bit))
            node = 2 * node + 1 + bit
        assert node == 31 + j
        paths.append(steps)
    return paths


def _pack_amat():
    """[128, 256] fp16: [:, :128] = A'dir, [:, 128:] = A'path, block-diagonal
    over 4 trees of (32 shallow nodes x 32 parents)."""
    adir = np.zeros((32, 32), np.float32)
    apath = np.zeros((32, 32), np.float32)
    for j, steps in enumerate(_parent_paths()):
        for node, bit in steps:
            if bit:
                adir[node, j] += 1.0
            apath[node, j] -= 1.0
    amat = np.zeros((128, 256), np.float16)
    for r in range(4):
        sl = slice(32 * r, 32 * r + 32)
        amat[sl, sl] = adir
        amat[sl, 128 + 32 * r:128 + 32 * r + 32] = apath
    return amat


_AMAT = _pack_amat()


def _build_bass():
    import concourse.bacc as bacc
    import concourse.mybir as mybir
    import concourse.tile as tile
    from concourse.hw_specs import get_activation_tables

    dt = mybir.dt
    f32 = dt.float32
    f32r = dt.float32r
    fp16 = dt.float16
    AF = mybir.ActivationFunctionType
    ALU = mybir.AluOpType

    nc = bacc.Bacc("TRN2", target_bir_lowering=False, debug=False,
                   num_devices=NCORES)

    table_id = next(i for i, (_, funcs) in
                    enumerate(get_activation_tables("gen3").items())
                    if AF.Exp in funcs and AF.Ln in funcs)
    nc.scalar.add_instruction(mybir.InstLoadActFuncSet(
        name=f"I-{nc.next_id()}", ins=[], outs=[], act_func_set_id=table_id))

    # ---- DRAM tensors ------------------------------------------------
    xt = nc.dram_tensor("xt", [128, 4096], fp16, kind="ExternalInput").ap()
    wt = nc.dram_tensor("wt", [4096, 256], fp16, kind="ExternalInput").ap()
    consts = nc.dram_tensor("consts", [128, 32], f32r,
                            kind="ExternalInput").ap()
    amat = nc.dram_tensor("amat", [128, 256], fp16, kind="ExternalInput").ap()
    vmat = nc.dram_tensor("vmat", [128, 2048], fp16,
                          kind="ExternalInput").ap()
    outs = {}
    for v in range(2):
        for h in ("A", "B"):
            nm = f"o{h}{v}"
            outs[(v, h)] = nc.dram_tensor(nm, [100, VB], f32,
                                          kind="ExternalOutput").ap()

    with tile.TileContext(nc) as tc:
        with (
            tc.tile_pool(name="big", bufs=1) as bigp,
            tc.tile_pool(name="const", bufs=1) as constp,
            tc.tile_pool(name="ta", bufs=3) as tap,
            tc.tile_pool(name="work", bufs=3) as work,
            tc.tile_pool(name="pz", bufs=2, space="PSUM") as pzp,
            tc.tile_pool(name="pp", bufs=2, space="PSUM") as ppp,
            tc.tile_pool(name="po", bufs=2, space="PSUM") as pop,
        ):
            wt_t = bigp.tile([128, 8192], fp16, tag="wt")
            xt_t = bigp.tile([128, 4096], fp16, tag="xt")
            vm_t = bigp.tile([128, 2048], fp16, tag="vm")
            consts_t = constp.tile([128, 32], f32r, tag="consts")
            amat_t = constp.tile([128, 256], fp16, tag="amat")
            warm_t = constp.tile([128, 128], fp16, tag="warm")

            # ---- warm-up: memset tile + back-to-back matmuls ---------
            nc.gpsimd.memset(warm_t[:], 0.0)
            warm_ps = pop.tile([128, VB], f32, tag="ops", name="warm_ps")
            for _ in range(N_WARM):
                nc.tensor.matmul(warm_ps[:, 0:128], lhsT=warm_t[:],
                                 rhs=warm_t[:], start=True, stop=True)

            # ---- DMA plan --------------------------------------------
            k = 0
            dma_engs = [nc.gpsimd, nc.sync, nc.scalar]

            def dma(out_, in_):
                nonlocal k
                dma_engs[k % len(dma_engs)].dma_start(out=out_, in_=in_)
                k += 1

            def wt_piece(j, g0, g1):
                dma(wt_t[:, j * 2048 + g0 * 256:j * 2048 + g1 * 256]
                    .rearrange("p (g c) -> p g c", c=256),
                    wt[j * 1024 + g0 * 128:j * 1024 + g1 * 128, :]
                    .rearrange("(g p) c -> p g c", p=128))

            def xt_piece(j, v, h0=0, h1=2):
                c0 = j * 1024 + v * VB
                dma(xt_t[:, c0 + h0 * 256:c0 + h1 * 256],
                    xt[:, c0 + h0 * 256:c0 + h1 * 256])

            dma(amat_t[:], amat[:])
            dma(consts_t[:], consts[:])
            # head: j-major so group-0/1 matmuls can start accumulating
            # as soon as each j-chunk of x and weights lands
            for j in range(4):
                xt_piece(j, 0, 0, 1)
                xt_piece(j, 0, 1, 2)
                wt_piece(j, 0, 2)
            # bulk: rest of weights, vmat, x v1
            wt_piece(0, 2, 5)
            wt_piece(1, 2, 5)
            dma(vm_t[:, 0:512], vmat[:, 0:512])
            wt_piece(2, 2, 5)
            wt_piece(3, 2, 5)
            dma(vm_t[:, 512:1024], vmat[:, 512:1024])
            wt_piece(0, 5, 8)
            wt_piece(1, 5, 8)
            dma(vm_t[:, 1024:2048], vmat[:, 1024:2048])
            wt_piece(2, 5, 8)
            wt_piece(3, 5, 8)
            for j in range(4):
                xt_piece(j, 1)

            adir_ap = amat_t[:, 0:128]
            apath_ap = amat_t[:, 128:256]

            def bias_ap(c):
                return consts_t[:, c:c + 1].bitcast(f32)

            ones_bc = (consts_t[:, 16:17].bitcast(f32)
                       .broadcast_to([128, 1024]))

            # per-(v,half) output accumulators; created lazily
            out_ps = {}
            osb_tiles = {}

            # ---- pipeline --------------------------------------------
            # unit = (v, g).  pairs of consecutive groups share ACT batches.
            state = {}   # pair -> dict of tiles

            def head(pair):
                v, ga = pair
                # ta2 layout: [sh_g | sh_g1 | dp_g | dp_g1]
                ta2 = tap.tile([128, 2048], fp16, tag="ta",
                               name=f"ta_{v}_{ga}")
                pzs = []
                for h, g in enumerate((ga, ga + 1)):
                    pz = pzp.tile([128, 1024], f32, tag="pz")
                    pzs.append(pz)
                    for half in range(2):
                        for j in range(4):
                            nc.tensor.matmul(
                                pz[:, half * VB:(half + 1) * VB],
                                lhsT=wt_t[:, j * 2048 + g * 256 + half * 128:
                                          j * 2048 + g * 256 + (half + 1) * 128],
                                rhs=xt_t[:, j * 1024 + v * VB:
                                         j * 1024 + (v + 1) * VB],
                                start=(j == 0), stop=(j == 3),
                            )
                # shallow evacs first so sp's inputs are ready earliest
                for h, g in enumerate((ga, ga + 1)):
                    nc.vector.tensor_scalar_add(
                        out=ta2[:, h * VB:(h + 1) * VB],
                        in0=pzs[h][:, 0:VB], scalar1=bias_ap(2 * g))
                for h, g in enumerate((ga, ga + 1)):
                    nc.vector.tensor_scalar_add(
                        out=ta2[:, 1024 + h * VB:1024 + (h + 1) * VB],
                        in0=pzs[h][:, VB:2 * VB], scalar1=bias_ap(2 * g + 1))
                te2 = work.tile([128, 2048], f32, tag="te",
                                name=f"te_{v}_{ga}")
                nc.scalar.activation(te2[:, 0:1024], ta2[:, 0:1024], AF.Exp)
                sp2 = work.tile([128, 1024], fp16, tag="sp",
                                name=f"sp_{v}_{ga}")
                nc.scalar.activation(sp2[:], te2[:, 0:1024], AF.Ln, bias=1.0)
                nc.scalar.activation(te2[:, 1024:2048], ta2[:, 1024:2048],
                                     AF.Exp)
                state[pair] = dict(ta2=ta2, sp2=sp2, te2=te2)

            def tail_a(pair, cw=VB):
                v, ga = pair
                st = state[pair]
                ta2, sp2, te2 = st["ta2"], st["sp2"], st["te2"]

                pps = [ppp.tile([128, VB], f32, tag="pp", name=f"pp{h}")
                       for h in range(2)]
                ep = work.tile([128, 1024], fp16, tag="ep",
                               name=f"ep_{v}_{ga}")
                for off in range(0, 1024, cw):
                    h, inner = off // VB, off % VB
                    psl = slice(inner, inner + cw)
                    esl = slice(off, off + cw)
                    nc.tensor.matmul(pps[h][:, psl], lhsT=adir_ap,
                                     rhs=ta2[:, esl],
                                     start=True, stop=False)
                    nc.tensor.matmul(pps[h][:, psl], lhsT=apath_ap,
                                     rhs=sp2[:, esl],
                                     start=False, stop=True)
                    nc.scalar.activation(ep[:, esl], pps[h][:, psl], AF.Exp)
                t1 = work.tile([128, 1024], f32, tag="t1",
                               name=f"t1_{v}_{ga}")
                nc.gpsimd.tensor_tensor(out=t1[:], in0=te2[:, 1024:2048],
                                        in1=ones_bc, op=ALU.add)
                r5 = work.tile([128, 1024], f32, tag="r5",
                               name=f"r5_{v}_{ga}")
                nc.vector.reciprocal_approx_fast(out=r5[:], in_=t1[:])
                st["ep"] = ep
                st["r5"] = r5

            def tail_b(pair, cw=VB):
                v, ga = pair
                st = state.pop(pair)
                ep, r5 = st["ep"], st["r5"]
                epr = work.tile([128, 1024], fp16, tag="epr",
                                name=f"epr_{v}_{ga}")
                half = "A" if ga < 4 else "B"
                if (v, half) not in out_ps:
                    out_ps[(v, half)] = pop.tile([128, VB], f32, tag="ops",
                                                 name=f"ops_{v}{half}")
                ops = out_ps[(v, half)]
                for off in range(0, 1024, cw):
                    g, inner = ga + off // VB, off % VB
                    esl = slice(off, off + cw)
                    bsl = slice(inner, inner + cw)
                    nc.gpsimd.tensor_tensor(out=epr[:, esl], in0=ep[:, esl],
                                            in1=r5[:, esl], op=ALU.mult)
                    nc.tensor.matmul(ops[:, bsl],
                                     lhsT=vm_t[:, g * 256:g * 256 + 128],
                                     rhs=ep[:, esl],
                                     start=(g % 4 == 0 and inner == 0),
                                     stop=False)
                    nc.tensor.matmul(ops[:, bsl],
                                     lhsT=vm_t[:, g * 256 + 128:g * 256 + 256],
                                     rhs=epr[:, esl],
                                     start=False,
                                     stop=(g % 4 == 3 and off == 1024 - cw))
                if ga + 1 in (3, 7):
                    osb = work.tile([128, VB], f32, tag=f"osb{half}",
                                    name=f"osb_{v}{half}")
                    nc.vector.tensor_copy(out=osb[:], in_=ops[:])
                    eng = nc.gpsimd if half == "A" else nc.sync
                    eng.dma_start(out=outs[(v, half)][:], in_=osb[0:100, :])
                    del out_ps[(v, half)]

            pairs = [(v, ga) for v in range(2) for ga in (0, 2, 4, 6)]
            for i, p in enumerate(pairs):
                head(p)
                if i >= 1:
                    tail_a(pairs[i - 1])
                if i >= 2:
                    tail_b(pairs[i - 2])
            tail_a(pairs[-1], cw=256)
            tail_b(pairs[-2])
            tail_b(pairs[-1], cw=256)

    nc.finalize()
    return nc


def _get_nc():
    if "nc" not in _NC_CACHE:
        _NC_CACHE["nc"] = _build_bass()
    return _NC_CACHE["nc"]


def _prep_inputs(x, split_weights, split_bias, leaf_logits, tree_weights):
    x = np.asarray(x, np.float32)
    split_weights = np.asarray(split_weights, np.float32)
    split_bias = np.asarray(split_bias, np.float32)
    leaf_logits = np.asarray(leaf_logits, np.float32)
    tree_weights = np.asarray(tree_weights, np.float32)

    w_soft = np.exp(tree_weights - tree_weights.max())
    w_soft = w_soft / w_soft.sum()

    # leaf distributions scaled by 2*w_t (host softmax)
    ll = leaf_logits - leaf_logits.max(axis=-1, keepdims=True)
    ev = np.exp(ll)
    dist = ev / ev.sum(axis=-1, keepdims=True)          # [T, L, C]
    vt = 2.0 * w_soft[:, None, None] * dist             # [T, L, C]

    in_maps = []
    for tg in range(TG):
        trees = np.arange(tg * TL, (tg + 1) * TL)
        # weights: wt_np[j, g, p, c]
        wt_np = np.zeros((4, NG, 128, 256), np.float32)
        bias_np = np.zeros((128, 32), np.float32)
        vm_np = np.zeros((128, NG, 2, 128), np.float32)
        for tl, t in enumerate(trees):
            g, r = tl // 4, tl % 4
            rows = slice(32 * r, 32 * r + 32)
            W = split_weights[t]                        # [N, D]
            bs = split_bias[t]                          # [N]
            # shallow nodes 0..30 (+pad), deep nodes 31..62
            wsh = np.zeros((32, D), np.float32)
            wsh[:31] = W[0:31]
            wdp = W[31:63]                              # [32, D]
            for j in range(4):
                dsl = slice(j * 128, (j + 1) * 128)
                wt_np[j, g, :, 32 * r:32 * r + 32] = wsh[:, dsl].T
                wt_np[j, g, :, 128 + 32 * r:128 + 32 * r + 32] = wdp[:, dsl].T
            bias_np[rows, 2 * g] = np.concatenate([bs[0:31], [0.0]])
            bias_np[rows, 2 * g + 1] = bs[31:63]
            # V_A = V_odd, V_B = V_even - V_odd  (parents j = 0..31)
            v_even = vt[t, 0::2, :]                     # [32, C]
            v_odd = vt[t, 1::2, :]
            vm_np[rows, g, 0, :C] = v_odd
            vm_np[rows, g, 1, :C] = v_even - v_odd
        wt16 = np.ascontiguousarray(
            wt_np.reshape(4096, 256).astype(np.float16))
        vm16 = np.ascontiguousarray(
            vm_np.reshape(128, 2048).astype(np.float16))
        bias_np[:, 16] = 1.0
        shared = dict(wt=wt16, consts=bias_np.copy(), amat=_AMAT, vmat=vm16)
        for bg in range(BG):
            xs = x[bg * BSL:(bg + 1) * BSL, :]          # [1024, 512]
            xt16 = np.ascontiguousarray(
                xs.T.reshape(4, 128, BSL).transpose(1, 0, 2)
                .reshape(128, 4096).astype(np.float16))
            in_maps.append(dict(xt=xt16, **shared))
    return in_maps


def kernel(x, split_weights, split_bias, leaf_logits, tree_weights):
    from concourse.bass_utils import run_bass_kernel_spmd

    in_maps = _prep_inputs(x, split_weights, split_bias, leaf_logits,
                           tree_weights)
    nc = _get_nc()
    res = run_bass_kernel_spmd(nc, in_maps, core_ids=list(range(NCORES)))
    out = np.zeros((B, C), np.float32)
    for tg in range(TG):
        for bg in range(BG):
            r = res.results[tg * BG + bg]
            for v in range(2):
                part = (r[f"oA{v}"] + r[f"oB{v}"]).T      # [512, 100]
                rows = slice(bg * BSL + v * VB, bg * BSL + (v + 1) * VB)
                out[rows] += part
    return np.ascontiguousarray(out)
